# revision 14
# baseline (speedup 1.0000x reference)
"""Trainium2 Bass kernel for nn_MHA_48120813584614 (dual cross-attention MHA).

Strategy (head/tensor parallel over 8 cores):
  - Core c owns head c for BOTH attention directions:
      pair 0 ("i"): metadata queries attend image keys/values  -> contributes to out_i
      pair 1 ("m"): image queries attend metadata keys/values -> contributes to out_m
  - All tensors kept transposed ([feature, token]) so no transposes are needed:
      QT[e,m] = WqT.T @ XTq ; KT[e,n] = WkT.T @ XTkv ; V[n,e] = XTkv.T @ WvT
      ST[n,m] = KT.T @ QT   (scores transposed: keys on partitions)
      E = exp(ST/sqrt(512)) (no max subtraction; logits are O(1) here)
      colsum[m] = ones.T @ E (softmax denominator via PE)
      OT[e,m]  = V.T @ E     (unnormalized attention output)
      partialT[o,m] = WlT.T @ OT ; then scaled by 1/colsum[m] (normalization
      commutes with the head-output linear)
  - Row-parallel output linear: per-head partials are ReduceScattered over the
    8 cores (one RS per direction; dir-i RS overlaps dir-m compute). Each core
    gets a 64-row shard of the transposed output, adds b_lin slice + residual
    slice, and returns it. Host concatenates + transposes.
  - All matmuls bf16 (fp32 PSUM accumulation); softmax denominator fp32.
"""

import sys

sys.path.insert(0, "/opt/trn_rl_repo")

import math

import ml_dtypes
import numpy as np

import concourse.bass as bass
import concourse.mybir as mybir
import concourse.tile as tile
from concourse import bacc
from concourse.bass_utils import run_bass_kernel_spmd

H = 8
D = 512
N = 2048
NCORES = 8
P = 128
MC = 512  # m-chunk (matmul free dim / PSUM bank)
NMC = N // MC  # 4
ET = D // P  # 4 e/d/o tiles
NT = N // P  # 16 n tiles
SCALE = 1.0 / math.sqrt(D)

bf16 = mybir.dt.bfloat16
f32 = mybir.dt.float32

AF = mybir.ActivationFunctionType


def _build(reps=1, single=False):
    ndev = 1 if single else NCORES
    nc = bacc.Bacc("TRN2", target_bir_lowering=False, debug=False, num_devices=ndev)

    def din(name, shape, dtype):
        return nc.dram_tensor(name, shape, dtype, kind="ExternalInput").ap()

    xt = [din("xt_i", [D, N], bf16), din("xt_m", [D, N], bf16)]
    wq = [din(f"wq{p}", [D, D], bf16) for p in range(2)]
    wk = [din(f"wk{p}", [D, D], bf16) for p in range(2)]
    wv = [din(f"wv{p}", [D, D], bf16) for p in range(2)]
    wl = [din(f"wl{p}", [D, D], bf16) for p in range(2)]
    bq = [din(f"bq{p}", [P, ET], f32) for p in range(2)]
    bk = [din(f"bk{p}", [P, ET], f32) for p in range(2)]
    bv = [din(f"bv{p}", [1, D], f32) for p in range(2)]
    ones128_d = din("ones128", [P, 1], f32)
    resid = [din("resid_i", [64, N], f32), din("resid_m", [64, N], f32)]
    blin = [din("blin_i", [64, 1], f32), din("blin_m", [64, 1], f32)]
    out_d = [
        nc.dram_tensor("out_i", [64, N], f32, kind="ExternalOutput").ap(),
        nc.dram_tensor("out_m", [64, N], f32, kind="ExternalOutput").ap(),
    ]

    with tile.TileContext(nc) as tc:
        with (
            tc.tile_pool(name="const", bufs=1) as cpool,
            tc.tile_pool(name="xt", bufs=1) as xtpool,
            tc.tile_pool(name="w", bufs=2) as wpool,
            tc.tile_pool(name="qkv", bufs=1) as qkvpool,
            tc.tile_pool(name="v", bufs=1) as vpool,
            tc.tile_pool(name="expst", bufs=2) as epool,
            tc.tile_pool(name="ot", bufs=2) as otpool,
            tc.tile_pool(name="small", bufs=2) as spool,
            tc.tile_pool(name="outsb", bufs=2) as opool,
            tc.tile_pool(name="post", bufs=1) as ppool,
            tc.tile_pool(name="psum", bufs=6, space="PSUM") as ps,
            tc.tile_pool(name="pscs", bufs=2, space="PSUM") as pscs,
            tc.tile_pool(name="dram", bufs=1, space="DRAM") as dr,
        ):
          for _rep in range(reps):
            # ---- constants / inputs ----
            xts = []
            for i in range(2):
                t = xtpool.tile([P, ET, N], bf16, tag=f"xt{i}")
                src = xt[i].rearrange("(t p) n -> p t n", p=P)
                for dt_ in range(ET):
                    nc.sync.dma_start(t[:, dt_, :], src[:, dt_, :])
                xts.append(t)
            ones128 = cpool.tile([P, 1], f32)
            nc.sync.dma_start(ones128[:], ones128_d[:])
            resid_sb = []
            blin_sb = []
            for i in range(2):
                rt = cpool.tile([64, N], f32, tag=f"resid{i}")
                nc.sync.dma_start(rt[:], resid[i][:])
                resid_sb.append(rt)
                bt = cpool.tile([64, 1], f32, tag=f"blin{i}")
                nc.sync.dma_start(bt[:], blin[i][:])
                blin_sb.append(bt)

            rs_in = [
                dr.tile([D, N], f32, tag=f"rsin{p}", name=f"rsin{p}") for p in range(2)
            ]
            rs_out = [
                dr.tile([64, N], f32, tag=f"rsout{p}", name=f"rsout{p}")
                for p in range(2)
            ]

            for p in range(2):
                xq = xts[1] if p == 0 else xts[0]  # query-side input (transposed)
                xkv = xts[0] if p == 0 else xts[1]  # key/value-side input

                # ---- per-pair weights ----
                wq_t = wpool.tile([P, ET, D], bf16, tag="wq")
                wk_t = wpool.tile([P, ET, D], bf16, tag="wk")
                wv_t = wpool.tile([P, ET, D], bf16, tag="wv")
                wl_t = wpool.tile([P, ET, D], bf16, tag="wl")
                for w_t, w_d in ((wq_t, wq[p]), (wk_t, wk[p]), (wv_t, wv[p]),
                                 (wl_t, wl[p])):
                    src = w_d.rearrange("(t p) e -> p t e", p=P)
                    for dt_ in range(ET):
                        nc.sync.dma_start(w_t[:, dt_, :], src[:, dt_, :])
                bq_t = wpool.tile([P, ET], f32, tag="bq")
                nc.sync.dma_start(bq_t[:], bq[p][:])
                bk_t = wpool.tile([P, ET], f32, tag="bk")
                nc.sync.dma_start(bk_t[:], bk[p][:])
                bv_t = wpool.tile([1, D], f32, tag="bv")
                nc.sync.dma_start(bv_t[:], bv[p][:])
                bv_bc = wpool.tile([P, D], f32, tag="bvbc")
                nc.gpsimd.partition_broadcast(bv_bc[:], bv_t[:])

                # ---- projections: QT/KT [e,tile][m], V [n,tile][e] ----
                qt_t = qkvpool.tile([P, ET, N], bf16, tag="qt")
                kt_t = qkvpool.tile([P, ET, N], bf16, tag="kt")
                v_t = vpool.tile([P, NT, D], bf16, tag="v")
                for eb in range(ET):
                    for mc in range(NMC):
                        psq = ps.tile([P, MC], f32, tag="ps")
                        for dt_ in range(ET):
                            nc.tensor.matmul(
                                psq[:],
                                wq_t[:, dt_, eb * P:(eb + 1) * P],
                                xq[:, dt_, mc * MC:(mc + 1) * MC],
                                start=(dt_ == 0),
                                stop=(dt_ == ET - 1),
                            )
                        nc.scalar.activation(
                            qt_t[:, eb, mc * MC:(mc + 1) * MC], psq[:],
                            AF.Identity, bias=bq_t[:, eb:eb + 1],
                        )
                        psk = ps.tile([P, MC], f32, tag="ps")
                        for dt_ in range(ET):
                            nc.tensor.matmul(
                                psk[:],
                                wk_t[:, dt_, eb * P:(eb + 1) * P],
                                xkv[:, dt_, mc * MC:(mc + 1) * MC],
                                start=(dt_ == 0),
                                stop=(dt_ == ET - 1),
                            )
                        nc.scalar.activation(
                            kt_t[:, eb, mc * MC:(mc + 1) * MC], psk[:],
                            AF.Identity, bias=bk_t[:, eb:eb + 1],
                        )
                for nt in range(NT):
                    psv = ps.tile([P, D], f32, tag="ps")
                    for dt_ in range(ET):
                        nc.tensor.matmul(
                            psv[:],
                            xkv[:, dt_, nt * P:(nt + 1) * P],
                            wv_t[:, dt_, :],
                            start=(dt_ == 0),
                            stop=(dt_ == ET - 1),
                        )
                    # bias add (broadcast along partitions) fused into the copy
                    nc.vector.tensor_tensor(
                        v_t[:, nt, :], psv[:], bv_bc[:], mybir.AluOpType.add
                    )

                # ---- attention + output linear, per m-chunk ----
                for mc in range(NMC):
                    e_t = epool.tile([P, NT, MC], bf16, tag="e")
                    acc = spool.tile([P, MC], f32, tag="acc")
                    for nt in range(NT):
                        pss = ps.tile([P, MC], f32, tag="ps")
                        for eb in range(ET):
                            nc.tensor.matmul(
                                pss[:],
                                kt_t[:, eb, nt * P:(nt + 1) * P],
                                qt_t[:, eb, mc * MC:(mc + 1) * MC],
                                start=(eb == 0),
                                stop=(eb == ET - 1),
                            )
                        nc.scalar.activation(e_t[:, nt, :], pss[:], AF.Exp, scale=SCALE)
                        # running per-partition sum of exp tiles (DVE)
                        if nt == 0:
                            nc.vector.tensor_copy(acc[:], e_t[:, 0, :])
                        else:
                            nc.vector.tensor_tensor(
                                acc[:], acc[:], e_t[:, nt, :], mybir.AluOpType.add
                            )
                    # softmax denominator: reduce acc over partitions (1 fp32 MM)
                    cs = pscs.tile([1, MC], f32, tag="cs")
                    nc.tensor.matmul(cs[:], ones128[:], acc[:], start=True, stop=True)
                    recip = spool.tile([1, MC], f32, tag="recip")
                    nc.vector.reciprocal(recip[:], cs[:])
                    rb = spool.tile([P, MC], f32, tag="rb")
                    nc.gpsimd.partition_broadcast(rb[:], recip[:])
                    # PV: OT[e, m] unnormalized
                    ot_t = otpool.tile([P, ET, MC], bf16, tag="ot")
                    for eb in range(ET):
                        pso = ps.tile([P, MC], f32, tag="ps")
                        for nt in range(NT):
                            nc.tensor.matmul(
                                pso[:],
                                v_t[:, nt, eb * P:(eb + 1) * P],
                                e_t[:, nt, :],
                                start=(nt == 0),
                                stop=(nt == NT - 1),
                            )
                        nc.vector.tensor_copy(ot_t[:, eb, :], pso[:])
                    # output linear partial + deferred softmax normalization
                    for ob in range(ET):
                        psl = ps.tile([P, MC], f32, tag="ps")
                        for eb in range(ET):
                            nc.tensor.matmul(
                                psl[:],
                                wl_t[:, eb, ob * P:(ob + 1) * P],
                                ot_t[:, eb, :],
                                start=(eb == 0),
                                stop=(eb == ET - 1),
                            )
                        res_sb = opool.tile([P, MC], f32, tag="res")
                        nc.vector.tensor_tensor(
                            res_sb[:], psl[:], rb[:], mybir.AluOpType.mult
                        )
                        nc.sync.dma_start(
                            rs_in[p][ob * P:(ob + 1) * P, mc * MC:(mc + 1) * MC],
                            res_sb[:],
                        )

                # ---- reduce-scatter partials over all 8 cores ----
                if not single:
                    nc.gpsimd.collective_compute(
                        "ReduceScatter",
                        mybir.AluOpType.add,
                        ins=[rs_in[p].opt()],
                        outs=[rs_out[p].opt()],
                        replica_groups=[list(range(NCORES))],
                    )
                po = ppool.tile([64, N], f32, tag="po")
                nc.sync.dma_start(po[:], rs_out[p][:] if not single else rs_in[p][0:64, :])
                nc.scalar.activation(po[:], po[:], AF.Identity, bias=blin_sb[p][:, 0:1])
                nc.vector.tensor_tensor(
                    po[:], po[:], resid_sb[p][:], mybir.AluOpType.add
                )
                nc.sync.dma_start(out_d[p][:], po[:])

    nc.compile()
    return nc


_NC_CACHE = {}


def _get_nc():
    if "nc" not in _NC_CACHE:
        _NC_CACHE["nc"] = _build()
    return _NC_CACHE["nc"]


def _make_in_maps(inputs):
    f = np.float32
    b = ml_dtypes.bfloat16

    def c_(x, dt):
        return np.ascontiguousarray(x).astype(dt)

    img = np.asarray(inputs["image_input"], f)
    meta = np.asarray(inputs["metadata_input"], f)
    xt_i = c_(img.T, b)
    xt_m = c_(meta.T, b)
    ones128 = np.ones((P, 1), f)

    in_maps = []
    for c in range(NCORES):
        m = {
            "xt_i": xt_i,
            "xt_m": xt_m,
            "ones128": ones128,
            "resid_i": c_(img[:, 64 * c:64 * (c + 1)].T, f),
            "resid_m": c_(meta[:, 64 * c:64 * (c + 1)].T, f),
            "blin_i": c_(np.asarray(inputs["b_lin_i"], f)[64 * c:64 * (c + 1)][:, None], f),
            "blin_m": c_(np.asarray(inputs["b_lin_m"], f)[64 * c:64 * (c + 1)][:, None], f),
        }
        for p, (Wq, bq_, Wk, bk_, Wv, bv_, Wl) in enumerate([
            (inputs["Wq_m"], inputs["bq_m"], inputs["Wk_i"], inputs["bk_i"],
             inputs["Wv_i"], inputs["bv_i"], inputs["W_lin_i"]),
            (inputs["Wq_i"], inputs["bq_i"], inputs["Wk_m"], inputs["bk_m"],
             inputs["Wv_m"], inputs["bv_m"], inputs["W_lin_m"]),
        ]):
            m[f"wq{p}"] = c_(np.asarray(Wq, f)[c].T, b)
            m[f"wk{p}"] = c_(np.asarray(Wk, f)[c].T, b)
            m[f"wv{p}"] = c_(np.asarray(Wv, f)[c].T, b)
            m[f"wl{p}"] = c_(np.asarray(Wl, f)[:, D * c:D * (c + 1)].T, b)
            m[f"bq{p}"] = c_(np.asarray(bq_, f)[c].reshape(ET, P).T, f)
            m[f"bk{p}"] = c_(np.asarray(bk_, f)[c].reshape(ET, P).T, f)
            m[f"bv{p}"] = c_(np.asarray(bv_, f)[c][None, :], f)
        in_maps.append(m)
    return in_maps


def _assemble(results):
    out_iT = np.concatenate([results[c]["out_i"] for c in range(NCORES)], axis=0)
    out_mT = np.concatenate([results[c]["out_m"] for c in range(NCORES)], axis=0)
    return np.concatenate([out_iT.T, out_mT.T], axis=1).astype(np.float32)


def kernel(**inputs):
    nc = _get_nc()
    in_maps = _make_in_maps(inputs)
    res = run_bass_kernel_spmd(nc, in_maps, list(range(NCORES)))
    return _assemble(res.results)


if __name__ == "__main__":
    _get_nc()
    print("build ok")
